# revision 40
# baseline (speedup 1.0000x reference)
"""Multi-head attention block (B=2, N=2048, C=1024, H=16) on 8 TRN2 NeuronCores.

Sharding (tensor-parallel over heads): core c owns global heads {2c, 2c+1}:
  - w_qkv columns for q/k/v of those heads  -> [1024, 384] slice
  - w_proj rows for those heads             -> [128, 1024] slice
  - x replicated, pre-transposed on host to xT [1024, 4096] (and cast bf16)
Each core computes a full [4096, 1024] partial of the output projection;
the host sums the 8 partials and adds b_proj.

Device pipeline per core (bf16 matmuls, fp32 PSUM accumulation):
  1. qkvT = w_slice.T @ xT -> qT/kT/vT in [head_dim, seq] layout (k/q/v
     groups emitted through a marker-gated background queue).
  2. Attention per (batch, 512-wide q chunk): both heads' scores^T
     [keys=128, 512] are packed into one [128, 1024] PSUM tile via
     row-group tile_position (the K=64 matmuls run concurrently in the
     PE array). Exp runs split across TWO engines: most key-chunks on
     ScalarE (1/sqrt(d) folded into the activation scale; no
     max-subtraction needed for these O(1) scores), and a few per chunk
     on the otherwise-lighter VectorE via a Schraudolph bit-trick
     (round(128*(log2e*s + 127 - c)) as uint16 == bf16 bits of exp(s),
     one tensor_scalar op). Then a V-matmul per head whose stationary
     operand is padded to the full 128 columns [v | ones | pad] - the
     ones column makes the PSUM accumulator also collect softmax
     denominators, and the full width keeps the PE HAM clock-gate at 8/8
     and enables fast weight loads. vps is drained to SBUF immediately
     (one [65, 512] copy per head: v-rows + denominator row);
     normalization (fast reciprocal + gpsimd partition_broadcast) runs
     off the critical path.
  3. out^T chunks feed the projection matmul directly as lhsT (k=128,
     no transpose); results stream out per [128, 512] tile, with output
     DMAs spread round-robin over the vector/gpsimd/tensor queues so the
     tail burst is not serialized behind one ring.
Scheduling: the exp stream is the pacing engine; qkv for later chunks,
V-weight builds, and projection chunks are interleaved into the
attention loop through the background queue to fill PE slack. A short
identity-matmul warmup keeps the PE clock-gate warm during initial DMAs.
"""

import math
import os

import numpy as np

os.environ.setdefault("JAX_PLATFORMS", "axon,cpu")

import concourse.mybir as mybir
import concourse.tile as tile
from concourse import bacc
from concourse.bass_utils import run_bass_kernel_spmd
from concourse.masks import make_identity

F32 = mybir.dt.float32
MMDT = mybir.dt.bfloat16  # matmul operand dtype
U16 = mybir.dt.uint16

# Problem shape (hardcoded per contract)
B, N, C, H = 2, 2048, 1024, 16
D = C // H            # 64 head dim
SEQ = B * N           # 4096
NCORES = 8
HL = H // NCORES      # 2 local heads per core
MW = 3 * HL * D       # 384 w_qkv slice cols (q|k|v for 2 heads)
KT = C // 128         # 8 contraction tiles for the projections
SC = 512              # seq chunk for qkv stage
NSC = SEQ // SC       # 8
KCN = N // 128        # 16 key chunks per batch
QW = 512              # q-chunk width for attention
NQH = N // QW         # 4
SCALE = 1.0 / math.sqrt(D)

# Schraudolph exp-on-DVE constants: bf16 bits of exp(s*SCALE) ~=
# round(128*(s*SCALE*log2e + 127 - c)); c tuned for min RMS error.
SCHR_A = 128.0 * math.log2(math.e) * SCALE
SCHR_B = 128.0 * (127.0 - 0.0434)
# key-chunks per 16 whose exp runs on VectorE instead of ScalarE
DVE_KCS = (5, 9, 13)


def build_nc():
    nc = bacc.Bacc("TRN2", target_bir_lowering=False, debug=False)
    xt_d = nc.dram_tensor("xt", [C, SEQ], MMDT, kind="ExternalInput")
    wqkv_d = nc.dram_tensor("wqkv", [C, MW], MMDT, kind="ExternalInput")
    wproj_d = nc.dram_tensor("wproj", [HL * D, C], MMDT, kind="ExternalInput")
    out_d = nc.dram_tensor("out", [SEQ, C], F32, kind="ExternalOutput")

    with tile.TileContext(nc) as tc:
        with (
            tc.tile_pool(name="const", bufs=1) as const,
            tc.tile_pool(name="qkvt", bufs=1) as qkvt,
            tc.tile_pool(name="vaugp", bufs=2) as vaugp,
            tc.tile_pool(name="ptp", bufs=3) as ptp,
            tc.tile_pool(name="ptdp", bufs=2) as ptdp,
            tc.tile_pool(name="outt", bufs=2) as outtp,
            tc.tile_pool(name="rp", bufs=2) as rp,
            tc.tile_pool(name="op", bufs=6) as op,
            tc.tile_pool(name="ps_st", bufs=2, space="PSUM") as ps_st,
            tc.tile_pool(name="ps_v", bufs=1, space="PSUM") as ps_v,
            tc.tile_pool(name="ps_aux", bufs=2, space="PSUM") as ps_aux,
        ):
            # ---- constants ----
            ident = const.tile([128, 128], MMDT, tag="ident")
            ones_sb = const.tile([128, 1], F32, tag="ones")
            w_sb = const.tile([128, KT, MW], MMDT, tag="wqkv")
            wp_sb = const.tile([128, C], MMDT, tag="wproj")
            # first data the pipeline needs, split across the sync + scalar
            # DMA rings so the head transfers run in parallel: k-columns of w
            # first (first scores need k then q; v can land later)
            wr = wqkv_d.ap().rearrange("(kt p) m -> p kt m", p=128)
            nc.sync.dma_start(w_sb[:, :, 128:256], wr[:, :, 128:256])  # k
            make_identity(nc, ident[:])
            nc.gpsimd.memset(ones_sb[:], 1.0)

            # HAM warmup: keep the PE busy during the initial DMA wait so the
            # clock gate is at 8/8 when real work lands
            wu = ps_aux.tile([128, SC], F32, tag="aux", name="wu")
            for _ in range(36):
                nc.tensor.matmul(wu[:, :128], ident[:], ident[:], start=True, stop=True)

            # persistent transposed qkv: [dim-of-2-heads=128, seq]
            q_sb = qkvt.tile([128, SEQ], MMDT, tag="q")
            k_sb = qkvt.tile([128, SEQ], MMDT, tag="k")
            v_sb = qkvt.tile([128, SEQ], MMDT, tag="v")
            dst = [q_sb, k_sb, v_sb]

            # full xT resident; per-chunk DMAs issued upfront (chunk 0 first,
            # then the projection weights, then the rest)
            xt_all = qkvt.tile([128, KT, SEQ], MMDT, tag="xt_all")

            def xt_dma(sc, eng=nc.sync):
                eng.dma_start(
                    xt_all[:, :, sc * SC : (sc + 1) * SC],
                    xt_d.ap()[:, sc * SC : (sc + 1) * SC].rearrange(
                        "(kt p) n -> p kt n", p=128
                    ),
                )

            # The whole input stream is HBM-port-bound and both rings share the
            # port with equal priority, so each tensor is SPLIT across the two
            # rings (both halves advance together = full port bandwidth per
            # tensor) and tensors are sequenced globally in consumption order:
            # xt0, w-k|w-q, w-v, xt1, xt2, xt3, wproj. A tiny dummy exp on
            # scalar forces the ACT table load before the exp stream.
            # Scalar-queue DMA instrs block the exp stream for ~2us each, so
            # the scalar ring carries ONLY pre-exp-stream transfers (xt0 half,
            # w-q). xt1-3 ride the gpsimd swdge ring in parallel; sync keeps
            # the rest of the critical set.
            xr0 = xt_d.ap()[:, :SC].rearrange("(kt p) n -> p kt n", p=128)
            nc.sync.dma_start(xt_all[:, :4, :SC], xr0[:, :4])
            nc.scalar.dma_start(xt_all[:, 4:, :SC], xr0[:, 4:])
            nc.scalar.dma_start(w_sb[:, :, :128], wr[:, :, :128])  # q
            dummy = const.tile([1, 2], F32, tag="dummy")
            nc.gpsimd.memset(dummy[:], 0.0)
            nc.scalar.activation(
                out=dummy[:, 1:], in_=dummy[:, :1],
                func=mybir.ActivationFunctionType.Exp, scale=1.0,
            )
            nc.sync.dma_start(w_sb[:, :, 256:], wr[:, :, 256:])  # v
            xt_dma(1, nc.gpsimd)
            xt_dma(2, nc.gpsimd)
            nc.sync.dma_start(wp_sb[:], wproj_d.ap())
            xt_dma(3, nc.gpsimd)
            # xt chunks for batch 1 are deferred into the background queue so
            # the early vaug xbar-transposes aren't stuck behind them on the
            # sync ring

            # ---- background queue with markers ----
            bg = []  # (key|None, closure)
            done = set()

            def pump(n=1):
                for _ in range(n):
                    if not bg:
                        return
                    key, fn = bg.pop(0)
                    fn()
                    if key is not None:
                        done.add(key)

            def pump_until(key):
                while key not in done:
                    k, fn = bg.pop(0)
                    fn()
                    if k is not None:
                        done.add(k)

            # ---- qkv + vaug emission (all via bg) ----
            def emit_qkv_part(holder, sc, m, part, nparts=4):
                if part == 0:
                    holder["ps"] = ps_aux.tile([128, SC], F32, tag="aux", name="qkv_ps")
                ps = holder["ps"]
                step = KT // nparts
                for kt in range(part * step, (part + 1) * step):
                    nc.tensor.matmul(
                        ps[:],
                        w_sb[:, kt, m * 128 : (m + 1) * 128],
                        xt_all[:, kt, sc * SC : (sc + 1) * SC],
                        start=(kt == 0),
                        stop=(kt == KT - 1),
                    )
                if part == nparts - 1:
                    nc.vector.tensor_copy(
                        out=dst[m][:, sc * SC : (sc + 1) * SC], in_=ps[:]
                    )

            vaug_store = {}

            def emit_vaug_tr(holder, b, h, piece):
                b0 = b * N
                va = vaugp.tile([128, 4, D + 2], MMDT, tag=f"vaug{h}_{piece}", name="va")
                if b == 0 and piece < 2:
                    # head: PE transposes (the xbar route would queue behind
                    # the input DMAs on the sync ring and starve the first AVs)
                    tr = ps_aux.tile([128, 4 * D], MMDT, tag="aux", name="tr")
                    for tt in range(4):
                        t = piece * 4 + tt
                        nc.tensor.transpose(
                            tr[:, tt * D : (tt + 1) * D],
                            v_sb[h * D : (h + 1) * D, b0 + t * 128 : b0 + (t + 1) * 128],
                            ident[h * D : (h + 1) * D, h * D : (h + 1) * D],
                        )
                    nc.vector.tensor_copy(
                        out=va[:, :, :D], in_=tr[:].rearrange("p (a d) -> p a d", a=4)
                    )
                else:
                    # steady state: v^T via the DMA xbar transpose engine
                    # (frees the PE): one [64, 512] -> [128, 4, 64] call;
                    # slab a holds keys a*128+p
                    stage = vaugp.tile(
                        [128, 4, D], MMDT, tag=f"vstage{h}", name="vstage"
                    )
                    nc.sync.dma_start_transpose(
                        stage[:],
                        v_sb[
                            h * D : (h + 1) * D,
                            b0 + piece * 512 : b0 + (piece + 1) * 512,
                        ],
                    )
                    nc.vector.tensor_copy(out=va[:, :, :D], in_=stage[:])
                nc.vector.tensor_copy(
                    out=va[:, :, D : D + 2],
                    in_=ones_sb[:, None, :].to_broadcast([128, 4, 2]),
                )
                vaug_store[(b, h, piece)] = va

            # queue per batch: per sc: k, q, v (scores need k+q first; vaug
            # only gates the V-matmul which lags the exp stream by a chunk)
            for b in range(B):
                if b == 1:
                    # batch-1 xt chunks ride the idle gpsimd ring so they don't
                    # congest sync (out-DMAs + vaug transposes live there)
                    for sc4 in range(NSC // 2, NSC):
                        bg.append((None, lambda sc4=sc4: xt_dma(sc4, nc.gpsimd)))
                for scl in range(NSC // 2):
                    sc = b * (NSC // 2) + scl
                    holder = {}
                    # scl0: k,q first (first scores gate on them); later scls:
                    # v before q (vaug pieces are needed a chunk earlier than
                    # the next q-range)
                    for m in ((1, 0, 2) if scl == 0 else (1, 2, 0)):
                        for part in range(4):
                            key = None
                            if part == 3:
                                key = ("kqv"[0 if m == 1 else 1 if m == 0 else 2], b, scl)
                            bg.append(
                                (
                                    key,
                                    lambda sc=sc, m=m, part=part, holder=holder: (
                                        emit_qkv_part(holder, sc, m, part)
                                    ),
                                )
                            )
                    for h in range(HL):
                        bg.append(
                            (
                                ("vaug", b, h, scl),
                                lambda b=b, h=h, scl=scl: emit_vaug_tr({}, b, h, scl),
                            )
                        )

            dma_engines = [nc.gpsimd, nc.sync]
            dma_rr = [0]

            def emit_proj_chunk(outt, b0, s2, nck, use_act=False):
                pp = ps_aux.tile([128, 512], F32, tag="aux", name="proj_ps")
                nc.tensor.matmul(
                    pp[:],
                    outt[:, s2 * 128 : (s2 + 1) * 128],
                    wp_sb[:, nck * 512 : (nck + 1) * 512],
                    start=True,
                    stop=True,
                )
                o_sb = op.tile([128, 512], F32, tag="o", name="o_sb")
                if use_act:
                    nc.scalar.copy(out=o_sb[:], in_=pp[:])
                    eng = nc.scalar
                else:
                    nc.vector.tensor_copy(out=o_sb[:], in_=pp[:])
                    eng = dma_engines[dma_rr[0] % len(dma_engines)]
                    dma_rr[0] += 1
                eng.dma_start(
                    out_d.ap()[
                        b0 + s2 * 128 : b0 + (s2 + 1) * 128,
                        nck * 512 : (nck + 1) * 512,
                    ],
                    o_sb[:],
                )

            # ---- attention ----
            def emit_scores(b, qh, kc):
                b0 = b * N
                q0 = b0 + qh * QW
                pump_until(("q", b, qh))
                pump_until(("k", b, kc // 4))
                st = ps_st.tile([128, 2 * QW], F32, tag="st", name="st")
                for h in range(HL):
                    hs = slice(h * D, (h + 1) * D)
                    nc.tensor.matmul(
                        st[:, h * QW : (h + 1) * QW],
                        k_sb[hs, b0 + kc * 128 : b0 + (kc + 1) * 128],
                        q_sb[hs, q0 : q0 + QW],
                        start=True,
                        stop=True,
                        tile_position=(h * D, 0),
                    )
                return st

            def exp_scalar(st):
                ptt = ptp.tile([128, 2 * QW], MMDT, tag="pt")
                nc.scalar.activation(
                    out=ptt[:],
                    in_=st[:],
                    func=mybir.ActivationFunctionType.Exp,
                    scale=SCALE,
                )
                return [ptt[:, h * QW : (h + 1) * QW] for h in range(HL)]

            def exp_dve(st):
                # Schraudolph exp on VectorE: uint16 bits == bf16 exp
                ptd = ptdp.tile([128, 2 * QW], U16, tag="ptd")
                nc.vector.tensor_scalar(
                    out=ptd[:],
                    in0=st[:],
                    scalar1=SCHR_A,
                    scalar2=SCHR_B,
                    op0=mybir.AluOpType.mult,
                    op1=mybir.AluOpType.add,
                )
                return [
                    ptd[:, h * QW : (h + 1) * QW].bitcast(MMDT) for h in range(HL)
                ]

            chunks = [(b, qh) for b in range(B) for qh in range(NQH)]
            pend = emit_scores(0, 0, 0)
            pend_pt = None  # pre-issued (DVE) exp for the pending st
            outt = outu = rs = None
            for ci, (b, qh) in enumerate(chunks):
                b0 = b * N
                if qh == 0:
                    outt = outtp.tile([128, N], MMDT, tag="outT", name="outt")
                    outu = [
                        outtp.tile([D, N], MMDT, tag=f"outu{h}", name="outu")
                        for h in range(HL)
                    ]
                    rs = [
                        rp.tile([1, N], F32, tag=f"r{h}", name="rs")
                        for h in range(HL)
                    ]
                vps = [
                    ps_v.tile([D + 2, QW], F32, tag=f"vps{h}", name=f"vps{h}")
                    for h in range(HL)
                ]
                last = ci == len(chunks) - 1
                qs = slice(qh * QW, (qh + 1) * QW)

                def emit_av(kc, pt_slices):
                    # V-matmul for tile kc, emitted one slot late so its exp
                    # has already finished (no PE head-of-line stall)
                    for h in range(HL):
                        pump_until(("vaug", b, h, kc // 4))
                        nc.tensor.matmul(
                            vps[h][:],
                            vaug_store[(b, h, kc // 4)][:, kc % 4, :],
                            pt_slices[h],
                            start=(kc == 0),
                            stop=(kc == KCN - 1),
                        )
                        if kc == KCN - 1 and not last:
                            # drain right after each head's final V-matmul so
                            # the next chunk's accumulators free up sooner
                            nc.vector.tensor_copy(
                                out=outu[h][:, qs], in_=vps[h][:D, :]
                            )
                            nc.vector.tensor_copy(
                                out=rs[h][:, qs], in_=vps[h][D : D + 1, :]
                            )
                        elif kc == KCN - 1:
                            # final chunk: only stage the denominators (on the
                            # now-idle ScalarE); the multiply reads vps directly
                            nc.scalar.copy(
                                out=rs[h][:, qs], in_=vps[h][D : D + 1, :]
                            )

                prev_av = None  # (kc, pt_slices) awaiting delayed AV emission
                for kc in range(KCN):
                    if kc + 1 < KCN:
                        nxt = emit_scores(b, qh, kc + 1)
                    elif ci + 1 < len(chunks):
                        nb, nqh = chunks[ci + 1]
                        nxt = emit_scores(nb, nqh, 0)
                    else:
                        nxt = None
                    if kc < KCN - 2:
                        pump(2 if len(bg) > 16 else 1)
                    # exp for the current tile: either pre-issued on DVE at the
                    # previous step, or on ScalarE now
                    pt_slices = pend_pt if pend_pt is not None else exp_scalar(pend)
                    pend_pt = None
                    # pre-issue next tile's exp on VectorE (one slot of lead so
                    # DVE queue latency stays off the PE critical path)
                    if nxt is not None and kc + 1 < KCN and (kc + 1) in DVE_KCS \
                            and ci > 0:
                        pend_pt = exp_dve(nxt)
                    if prev_av is not None:
                        emit_av(*prev_av)
                    prev_av = (kc, pt_slices)
                    pend = nxt
                emit_av(*prev_av)

                # normalize off the critical path; queue this q-range's
                # projection chunks as background work
                def norm_and_proj(
                    b0=b0, qh=qh, outt=outt, outu=outu, rs=rs, b=b,
                    vps=vps, last=last
                ):
                    if last:
                        # keep the PE clock-gate warm through the serial norm
                        # chain so the final projection matmuls run at 2.4 GHz
                        wk = ps_aux.tile([128, SC], F32, tag="aux", name="warm")
                        for _ in range(32):
                            nc.tensor.matmul(
                                wk[:, :128], ident[:], ident[:],
                                start=True, stop=True,
                            )
                    for h in range(HL):
                        qs = slice(qh * QW, (qh + 1) * QW)
                        rb = rp.tile([D, QW], F32, tag="rb", name="rb")
                        nc.gpsimd.partition_broadcast(rb[:], rs[h][:, qs])
                        rbr = rp.tile([D, QW], F32, tag="rbr", name="rbr")
                        nc.vector.reciprocal_approx_fast(out=rbr[:], in_=rb[:])
                        nc.vector.tensor_mul(
                            out=outt[h * D : (h + 1) * D, qs],
                            in0=vps[h][:D, :] if last else outu[h][:, qs],
                            in1=rbr[:],
                        )
                    tail = last
                    for s2 in range(qh * (QW // 128), (qh + 1) * (QW // 128)):
                        for nck in range(C // 512):
                            ua = tail and (s2 + nck) % 2 == 1
                            bg.append(
                                (
                                    None,
                                    lambda outt=outt, b0=b0, s2=s2, nck=nck, ua=ua: (
                                        emit_proj_chunk(outt, b0, s2, nck, ua)
                                    ),
                                )
                            )

                bg.append((None, norm_and_proj))

            # drain remaining background work
            while bg:
                pump(1)
    nc.compile()
    return nc


_NC_CACHE = {}


def _get_nc():
    if "nc" not in _NC_CACHE:
        _NC_CACHE["nc"] = build_nc()
    return _NC_CACHE["nc"]


def make_in_maps(x, w_qkv, w_proj):
    np_dt = mybir.dt.np(MMDT)
    x = np.asarray(x, dtype=np.float32)
    w_qkv = np.asarray(w_qkv, dtype=np.float32)
    w_proj = np.asarray(w_proj, dtype=np.float32)
    xt = np.ascontiguousarray(x.reshape(SEQ, C).T.astype(np_dt))
    in_maps = []
    for c in range(NCORES):
        cs = slice(128 * c, 128 * c + 128)
        wslice = np.ascontiguousarray(
            np.concatenate(
                [w_qkv[:, cs], w_qkv[:, C:][:, cs], w_qkv[:, 2 * C :][:, cs]], axis=1
            ).astype(np_dt)
        )
        in_maps.append(
            {
                "xt": xt,
                "wqkv": wslice,
                "wproj": np.ascontiguousarray(w_proj[cs, :].astype(np_dt)),
            }
        )
    return in_maps


def kernel(x, w_qkv, w_proj, b_proj, _run_kwargs=None):
    # snapshot inputs to host numpy before any device/compile interaction
    in_maps = make_in_maps(x, w_qkv, w_proj)
    b_proj = np.asarray(b_proj, dtype=np.float32)
    nc = _get_nc()
    res = run_bass_kernel_spmd(
        nc, in_maps, core_ids=list(range(NCORES)), **(_run_kwargs or {})
    )
    acc = res.results[0]["out"].astype(np.float32)
    for c in range(1, NCORES):
        acc = acc + res.results[c]["out"]
    acc = acc + np.asarray(b_proj, dtype=np.float32)[None, :]
    out = acc.reshape(B, N, C)
    if _run_kwargs:
        kernel.last_result = res
    return out
